# revision 26
# baseline (speedup 1.0000x reference)
# Trainium2 Bass kernel for DeepFeatureKNN: exact k-NN (k<=16) of 4096 queries
# against 65536 database embeddings (D=256), sharded over 8 NeuronCores.
#
# Device algorithm (per core, database shard of 8192 rows):
#   psum[q, n] = 2*dots[q, n] - e2[n]          (PE: 2x float32r K=128 matmuls
#                                               + 1 K=3 bf16 matmul folding -e2
#                                               as an outer product with ones)
#   v tile     = copy(psum)                     (ACT engine, PSUM -> SBUF)
#   per 512-column tile: top-8 values + indices (DVE max / max_index)
# Host: merge 8 cores x 16 tiles x 8 candidates per query, exactness
# certificate (a tile can hide a top-k element only if its 8th-best beats the
# merged k-th best), numpy re-scan fallback for any uncertified query, then
# gather embedding rows.
#
# v = 2*dots - e2 = s2 - dist orders identically to -dist per query (s2 is
# constant per query), so descending v == ascending distance.

import os
import numpy as np
import ml_dtypes

N, D, MQ = 65536, 256, 4096
NCORES = 8
NSH = N // NCORES       # 8192 database rows per core
QT = 128                # queries per partition tile
NT = 512                # database columns per tile (= one PSUM bank of fp32)
NTILES = NSH // NT      # 16
QTILES = MQ // QT       # 32
KMAX = 16

_CACHE = {}


def _build_bass(mm_dtype: str):
    import concourse.bacc as bacc
    import concourse.tile as tile
    import concourse.mybir as mybir

    f32 = mybir.dt.float32
    bf16 = mybir.dt.bfloat16
    u16 = mybir.dt.uint16

    nc = bacc.Bacc("TRN2", target_bir_lowering=False, debug=False,
                   num_devices=NCORES)

    f16 = mybir.dt.float16
    if mm_dtype in ("bf16x3", "fp16x3"):
        # hi/lo half-precision operand planes: emb_hi/emb_lo, sub_hi/sub_lo
        hdt = bf16 if mm_dtype == "bf16x3" else f16
        embT_hi_d = nc.dram_tensor("embT_hi", [2, 128, NSH], hdt, kind="ExternalInput")
        embT_lo_d = nc.dram_tensor("embT_lo", [2, 128, NSH], hdt, kind="ExternalInput")
        subT_hi_d = nc.dram_tensor("subT_hi", [2, 128, MQ], hdt, kind="ExternalInput")
        subT_lo_d = nc.dram_tensor("subT_lo", [2, 128, MQ], hdt, kind="ExternalInput")
    else:
        embT_d = nc.dram_tensor("embT", [2, 128, NSH], f32, kind="ExternalInput")
        subT_d = nc.dram_tensor("subT2", [2, 128, MQ], f32, kind="ExternalInput")
    e2hl_d = nc.dram_tensor("e2hl", [3, NSH], bf16, kind="ExternalInput")
    cand_v_d = nc.dram_tensor("cand_v", [MQ, NTILES * 8], f32, kind="ExternalOutput")
    cand_i_d = nc.dram_tensor("cand_i", [MQ, NTILES * 8], u16, kind="ExternalOutput")

    with tile.TileContext(nc) as tc:
        with (
            tc.tile_pool(name="const", bufs=1) as const,
            tc.tile_pool(name="vpool", bufs=4) as vpool,
            tc.tile_pool(name="candp", bufs=3) as candp,
            tc.tile_pool(name="ps", bufs=8, space="PSUM") as psum,
        ):
            if mm_dtype in ("bf16x3", "fp16x3"):
                hdt = bf16 if mm_dtype == "bf16x3" else f16
                embT_hi = const.tile([128, 2, NSH], hdt)
                embT_lo = const.tile([128, 2, NSH], hdt)
                subT_hi = const.tile([128, 2, MQ], hdt)
                subT_lo = const.tile([128, 2, MQ], hdt)
                for c in range(2):
                    nc.gpsimd.dma_start(embT_hi[:, c, :], embT_hi_d[c])
                    nc.gpsimd.dma_start(embT_lo[:, c, :], embT_lo_d[c])
                    nc.gpsimd.dma_start(subT_hi[:, c, :], subT_hi_d[c])
                    nc.gpsimd.dma_start(subT_lo[:, c, :], subT_lo_d[c])
            elif mm_dtype == "f32r":
                # FP32r matmul operands must be produced by a compute op that
                # rounds to the fp32r packing; DMA alone won't do. Stage fp32
                # in chunks and round into the resident fp32r tiles.
                embT = const.tile([128, 2, NSH], mybir.dt.float32r)
                subT = const.tile([128, 2, MQ], mybir.dt.float32r)
                PCe = min(2048, NSH)
                PCs = min(2048, MQ)
                with tc.tile_pool(name="stage", bufs=3) as stage:
                    for c in range(2):
                        for p in range(NSH // PCe):
                            sl = slice(p * PCe, (p + 1) * PCe)
                            tmp = stage.tile([128, PCe], f32, tag="tmp",
                                             padded_shape=[128, 2048])
                            nc.gpsimd.dma_start(tmp[:], embT_d[c, :, sl])
                            nc.vector.tensor_copy(embT[:, c, sl], tmp[:])
                        for p in range(MQ // PCs):
                            sl = slice(p * PCs, (p + 1) * PCs)
                            tmp = stage.tile([128, PCs], f32, tag="tmp",
                                             padded_shape=[128, 2048])
                            nc.gpsimd.dma_start(tmp[:], subT_d[c, :, sl])
                            nc.vector.tensor_copy(subT[:, c, sl], tmp[:])
            else:
                embT = const.tile([128, 2, NSH], f32)
                subT = const.tile([128, 2, MQ], f32)
                for c in range(2):
                    nc.gpsimd.dma_start(embT[:, c, :], embT_d[c])
                    nc.gpsimd.dma_start(subT[:, c, :], subT_d[c])
            e2hl = const.tile([3, NSH], bf16)
            nc.gpsimd.dma_start(e2hl[:], e2hl_d[:])
            ones3 = const.tile([3, 128], bf16)
            nc.gpsimd.memset(ones3[:], 1.0)

            repeat = int(os.environ.get("KNN_REPEAT", "1"))
            for _rep in range(repeat):
                _build_loop(nc, tc, mybir, mm_dtype, locals())
    nc.compile()
    return nc


def _build_loop(nc, tc, mybir, mm_dtype, env):
    f32 = mybir.dt.float32
    u16 = mybir.dt.uint16
    vpool, candp, psum = env["vpool"], env["candp"], env["psum"]
    e2hl, ones3 = env["e2hl"], env["ones3"]
    cand_v_d, cand_i_d = env["cand_v_d"], env["cand_i_d"]
    if mm_dtype in ("bf16x3", "fp16x3"):
        embT_hi, embT_lo = env["embT_hi"], env["embT_lo"]
        subT_hi, subT_lo = env["subT_hi"], env["subT_lo"]
    else:
        embT, subT = env["embT"], env["subT"]
    if True:
            for qt in range(QTILES):
                cv = candp.tile([128, NTILES, 8], f32, tag="cv")
                ci = candp.tile([128, NTILES, 8], u16, tag="ci")
                for nt in range(NTILES):
                    ps = psum.tile([128, NT], f32, tag="ps")
                    qs = slice(qt * QT, (qt + 1) * QT)
                    ns = slice(nt * NT, (nt + 1) * NT)
                    if mm_dtype in ("bf16x3", "fp16x3"):
                        # dots2 ~= hi*hi + hi*lo + lo*hi  (both operands
                        # pre-scaled so psum accumulates 2*dots)
                        nc.tensor.matmul(ps[:], subT_hi[:, 0, qs], embT_hi[:, 0, ns],
                                         start=True, stop=False)
                        nc.tensor.matmul(ps[:], subT_hi[:, 1, qs], embT_hi[:, 1, ns],
                                         start=False, stop=False)
                        nc.tensor.matmul(ps[:], subT_hi[:, 0, qs], embT_lo[:, 0, ns],
                                         start=False, stop=False)
                        nc.tensor.matmul(ps[:], subT_hi[:, 1, qs], embT_lo[:, 1, ns],
                                         start=False, stop=False)
                        nc.tensor.matmul(ps[:], subT_lo[:, 0, qs], embT_hi[:, 0, ns],
                                         start=False, stop=False)
                        nc.tensor.matmul(ps[:], subT_lo[:, 1, qs], embT_hi[:, 1, ns],
                                         start=False, stop=False)
                    else:
                        nc.tensor.matmul(ps[:], subT[:, 0, qs], embT[:, 0, ns],
                                         start=True, stop=False)
                        nc.tensor.matmul(ps[:], subT[:, 1, qs], embT[:, 1, ns],
                                         start=False, stop=False)
                    # psum += ones^T @ (-e2 split rows)
                    nc.tensor.matmul(ps[:], ones3[:], e2hl[:, ns],
                                     start=False, stop=True)

                    v = vpool.tile([128, NT], f32, tag="v")
                    nc.scalar.activation(v[:], ps[:],
                                         mybir.ActivationFunctionType.Copy)
                    nc.vector.max(cv[:, nt, :], v[:])
                    nc.vector.max_index(ci[:, nt, :], cv[:, nt, :], v[:])
                nc.gpsimd.dma_start(cand_v_d[qt * QT:(qt + 1) * QT, :], cv[:])
                nc.gpsimd.dma_start(cand_i_d[qt * QT:(qt + 1) * QT, :], ci[:])


def _prep_inputs(emb: np.ndarray, sub: np.ndarray, mm_dtype: str):
    bf16 = ml_dtypes.bfloat16
    # -e2 as a 3-term bf16 cascade (repr error ~4e-6)
    e2 = -(emb.astype(np.float64) ** 2).sum(-1).astype(np.float32)  # [N]
    t0 = e2.astype(bf16)
    r0 = e2 - t0.astype(np.float32)
    t1 = r0.astype(bf16)
    r1 = r0 - t1.astype(np.float32)
    t2 = r1.astype(bf16)
    e2hl = np.stack([t0, t1, t2]).reshape(3, NCORES, NSH)   # [3, 8, 8192]

    maps = []
    if mm_dtype in ("bf16x3", "fp16x3"):
        hdt = bf16 if mm_dtype == "bf16x3" else np.float16
        s = np.sqrt(2.0, dtype=np.float32)
        embs = emb * s
        subs = sub * s
        ehi = embs.astype(hdt)
        elo = (embs - ehi.astype(np.float32)).astype(hdt)
        shi = subs.astype(hdt)
        slo = (subs - shi.astype(np.float32)).astype(hdt)
        ehiT = np.ascontiguousarray(ehi.T.reshape(2, 128, N))
        eloT = np.ascontiguousarray(elo.T.reshape(2, 128, N))
        shiT = np.ascontiguousarray(shi.T.reshape(2, 128, MQ))
        sloT = np.ascontiguousarray(slo.T.reshape(2, 128, MQ))
        for c in range(NCORES):
            sl = slice(c * NSH, (c + 1) * NSH)
            maps.append({
                "embT_hi": np.ascontiguousarray(ehiT[:, :, sl]),
                "embT_lo": np.ascontiguousarray(eloT[:, :, sl]),
                "subT_hi": shiT, "subT_lo": sloT,
                "e2hl": np.ascontiguousarray(e2hl[:, c]),
            })
    else:
        embT = np.ascontiguousarray(emb.T.reshape(2, 128, N))
        subT2 = np.ascontiguousarray((2.0 * sub).T.reshape(2, 128, MQ))
        for c in range(NCORES):
            maps.append({
                "embT": np.ascontiguousarray(embT[:, :, c * NSH:(c + 1) * NSH]),
                "subT2": subT2,
                "e2hl": np.ascontiguousarray(e2hl[:, c]),
            })
    return maps


def _device_candidates(emb: np.ndarray, sub: np.ndarray, mm_dtype: str):
    from concourse.bass_utils import run_bass_kernel_spmd
    key = ("nc", mm_dtype)
    if key not in _CACHE:
        _CACHE[key] = _build_bass(mm_dtype)
    nc = _CACHE[key]
    in_maps = _prep_inputs(emb, sub, mm_dtype)
    trace = bool(int(os.environ.get("KNN_TRACE", "0")))
    try:
        res = run_bass_kernel_spmd(nc, in_maps, core_ids=list(range(NCORES)),
                                   trace=trace)
    except ModuleNotFoundError:
        # axon NTFF profiling hook unavailable in this container
        res = run_bass_kernel_spmd(nc, in_maps, core_ids=list(range(NCORES)),
                                   trace=False)
    _CACHE["last_result"] = res
    vals = np.stack([r["cand_v"] for r in res.results])          # [8, MQ, 128]
    lidx = np.stack([r["cand_i"] for r in res.results])          # [8, MQ, 128]
    return vals, lidx


def _merge_host(emb, sub, vals, lidx, k):
    # global index: core*8192 + tile*512 + local
    tile_base = (np.arange(NTILES, dtype=np.int64) * NT).repeat(8)  # [128]
    gidx = (np.arange(NCORES, dtype=np.int64)[:, None, None] * NSH
            + tile_base[None, None, :] + lidx.astype(np.int64))     # [8,MQ,128]
    V = np.concatenate([vals[c] for c in range(NCORES)], axis=1)    # [MQ, 1024]
    G = np.concatenate([gidx[c] for c in range(NCORES)], axis=1)    # [MQ, 1024]
    _CACHE["last_vals"] = V
    _CACHE["last_gidx"] = G

    # top-(k+1) by v (descending v == ascending distance)
    part = np.argpartition(-V, [k - 1, k], axis=1)
    sel = part[:, :k]                                               # [MQ, k]
    topv = np.take_along_axis(V, sel, axis=1)
    topi = np.take_along_axis(G, sel, axis=1)
    v16 = topv.min(axis=1)
    v17 = np.take_along_axis(V, part[:, k:k + 1], axis=1)[:, 0]

    # Exactness certificate 1: the weakest shipped candidate of every 512-tile
    # (slot 7) must not beat the merged k-th best; otherwise that tile could
    # hide an unshipped better element.
    w8 = V.reshape(MQ, NCORES * NTILES, 8)[:, :, 7]                 # [MQ, 128]
    bad1 = (w8 > v16[:, None]).any(axis=1)

    # Certificate 2 (close call): if the margin between the k-th and (k+1)-th
    # best candidate is within the device GEMM error, the set membership at
    # the boundary is not trustworthy -> recompute those queries exactly.
    bad2 = (v16 - v17) < 2e-3
    bad = np.nonzero(bad1 | bad2)[0]
    _CACHE["last_bad"] = bad
    if bad.size:
        # exact set rescue in fp64 (true nearest neighbours)
        e2 = (emb.astype(np.float64) ** 2).sum(-1)
        emb64 = emb.astype(np.float64)
        for q in bad:
            dist = e2 - 2.0 * (emb64 @ sub[q].astype(np.float64))
            topi[q] = np.argpartition(dist, k)[:k]

    # Final ordering: rank the selected candidates by their fp64 (true)
    # distances — the statistically optimal order against any fp32-accurate
    # reference; stable sort with index-ascending base order matches
    # jax.lax.top_k tie handling.
    topi = np.sort(topi, axis=1)                  # idx-ascending base order
    cand = emb[topi].astype(np.float64)                            # [MQ,k,D]
    e2d = (cand * cand).sum(-1)
    d64 = e2d - 2.0 * np.einsum("qkd,qd->qk", cand, sub.astype(np.float64))
    order = np.argsort(d64, axis=1, kind="stable")
    topi = np.take_along_axis(topi, order, axis=1)
    _CACHE["last_topi"] = topi
    return topi


def kernel(embeddings, subset, k):
    emb = np.ascontiguousarray(np.asarray(embeddings, dtype=np.float32))
    sub = np.ascontiguousarray(np.asarray(subset, dtype=np.float32))
    kk = int(np.asarray(k))
    if emb.shape != (N, D) or sub.shape != (MQ, D) or not (1 <= kk <= KMAX):
        # off-spec shapes: exact numpy fallback
        e2 = (emb * emb).sum(-1)
        dist = e2[:, None] - 2.0 * (emb @ sub.T)
        idx = np.argsort(dist, axis=0, kind="stable")[:kk].T
        return emb[idx]

    mm_dtype = os.environ.get("KNN_MM_DTYPE", "fp16x3")
    vals, lidx = _device_candidates(emb, sub, mm_dtype)
    topi = _merge_host(emb, sub, vals, lidx, kk)
    return emb[topi]
